# revision 6
# baseline (speedup 1.0000x reference)
"""DyGrEncoder (GatedGraphConv x3 + GRUCell + LSTM) as a Bass/Tile SPMD kernel
on 8 TRN2 NeuronCores.

Sharding: nodes row-wise across 8 cores. Per conv layer: local m = h @ W,
AllGather m (bf16) across cores, per-edge indirect-DMA gather of source rows
(edges pre-sorted by destination block on the host), weighted segment-sum via
accumulating one-hot matmuls into PSUM (fp32), GRU update in fp32
(node-parallel). Final single-step LSTM is node-parallel. Node tensors live
on-chip transposed [128 feat, nodes] so every matmul is feature-contracted
with nodes streaming on the free dimension.

Precision: fp32 everywhere except the gathered-message path (m, AllGather,
msg, S -- bf16) and the LSTM hidden/cell inputs (bf16).
"""
import os
import numpy as np
import ml_dtypes

import concourse.bass as bass
import concourse.mybir as mybir
import concourse.tile as tile
from concourse import bacc
from concourse.bass_utils import run_bass_kernel_spmd

P = 128
NCORES = 8
f32 = mybir.dt.float32
bf16 = mybir.dt.bfloat16
i32 = mybir.dt.int32
AF = mybir.ActivationFunctionType
ALU = mybir.AluOpType
BF = ml_dtypes.bfloat16


# ----------------------------------------------------------------- host side

def _preprocess_edges(edge_index, edge_weight, N, NL, NB):
    """Sort each core's incoming edges by destination block; pad each block's
    edge list to cap[j]*128 (cap shared across cores so the SPMD program is
    identical). Returns per-core packed [128, ncols] arrays + cap."""
    src = np.asarray(edge_index[0]).astype(np.int64)
    dst = np.asarray(edge_index[1]).astype(np.int64)
    w = np.asarray(edge_weight).astype(np.float32)

    per_core = []
    counts = np.zeros((NCORES, NB), dtype=np.int64)
    for r in range(NCORES):
        lo, hi = r * NL, (r + 1) * NL
        m = (dst >= lo) & (dst < hi)
        es, ed, ew = src[m], dst[m] - lo, w[m]
        order = np.argsort(ed, kind='stable')
        es, ed, ew = es[order], ed[order], ew[order]
        counts[r] = np.bincount(ed // 128, minlength=NB)
        per_core.append((es, ed, ew))

    cap = np.maximum(np.ceil(counts / 128).astype(np.int64).max(axis=0), 1)
    ncols = int(cap.sum())

    out = []
    for r in range(NCORES):
        es, ed, ew = per_core[r]
        src_idx = np.zeros(ncols * 128, dtype=np.int32)
        slot = np.zeros(ncols * 128, dtype=np.float32)
        wgt = np.zeros(ncols * 128, dtype=np.float32)
        pos = 0
        start = 0
        for j in range(NB):
            cnt = int(counts[r, j])
            seg = slice(start, start + cnt)
            src_idx[pos:pos + cnt] = es[seg]
            slot[pos:pos + cnt] = (ed[seg] - j * 128).astype(np.float32)
            wgt[pos:pos + cnt] = ew[seg]
            pos += int(cap[j]) * 128
            start += cnt
        # pack edge i = c*128+p at [p, c]
        out.append(dict(
            esrc=np.ascontiguousarray(src_idx.reshape(ncols, 128).T),
            eslot=np.ascontiguousarray(slot.reshape(ncols, 128).T),
            ew=np.ascontiguousarray(wgt.reshape(ncols, 128).T),
        ))
    return out, cap, ncols


def _padT(a, NLP, dt=np.float32):
    """[n, D] float -> [D, NLP] padded transpose."""
    aT = np.ascontiguousarray(np.asarray(a).T.astype(np.float32))
    out = np.zeros((aT.shape[0], NLP), dtype=np.float32)
    out[:, :aT.shape[1]] = aT
    return out.astype(dt)


# ---------------------------------------------------------------- bass build

def _build(N, D, L, NL, NB, NLP, cap, ncols):
    nc = bacc.Bacc("TRN2", target_bir_lowering=False, debug=False,
                   num_devices=NCORES)
    dp = nc.declare_dram_parameter

    hT0_in = dp("hT0", [P, NLP], f32, isOutput=False)
    HT_in = dp("HT", [P, NLP], bf16, isOutput=False)
    CT_in = dp("CT", [P, NLP], bf16, isOutput=False)
    convW_in = dp("convW", [P, L * P], f32, isOutput=False)
    gWih_in = dp("gWihT", [P, 3 * P], f32, isOutput=False)
    gWhh_in = dp("gWhhT", [P, 3 * P], f32, isOutput=False)
    grub_in = dp("grub", [P, 4], f32, isOutput=False)
    lWih_in = dp("lWihT", [P, 4 * P], bf16, isOutput=False)
    lWhh_in = dp("lWhhT", [P, 4 * P], bf16, isOutput=False)
    lstmb_in = dp("lstmb", [P, 4], f32, isOutput=False)
    esrc_in = dp("esrc", [P, ncols], i32, isOutput=False)
    eslot_in = dp("eslot", [P, ncols], f32, isOutput=False)
    ew_in = dp("ew", [P, ncols], f32, isOutput=False)
    iota_in = dp("iota", [P, P], bf16, isOutput=False)
    Hout_ext = dp("HoutT", [P, NLP], f32, isOutput=True)
    Cout_ext = dp("CoutT", [P, NLP], f32, isOutput=True)

    lastw = NL - (NB - 1) * P          # rows in the last (partial) block
    chunks = [(s, min(512, NLP - s)) for s in range(0, NLP, 512)]

    with tile.TileContext(nc) as tc:
        with (
            tc.tile_pool(name="dram", bufs=1, space="DRAM") as dram,
            tc.tile_pool(name="persist", bufs=1) as pers,
            tc.tile_pool(name="msgp", bufs=8) as msgp,
            tc.tile_pool(name="sp", bufs=8) as sp,
            tc.tile_pool(name="tmp", bufs=2) as tp,
            tc.tile_pool(name="pagg", bufs=4, space="PSUM") as pagg,
            tc.tile_pool(name="pbig", bufs=4, space="PSUM") as pbig,
        ):
            # ---- persistent SBUF state
            hT = pers.tile([P, NLP], f32, name="hT")
            convW = pers.tile([P, L * P], f32, name="convW")
            gWih = pers.tile([P, 3 * P], f32, name="gWih")
            gWhh = pers.tile([P, 3 * P], f32, name="gWhh")
            grub = pers.tile([P, 4], f32, name="grub")
            lWih = pers.tile([P, 4 * P], bf16, name="lWih")
            lWhh = pers.tile([P, 4 * P], bf16, name="lWhh")
            lstmb = pers.tile([P, 4], f32, name="lstmb")
            esrc = pers.tile([P, ncols], i32, name="esrc")
            eslot = pers.tile([P, ncols], f32, name="eslot")
            ew = pers.tile([P, ncols], f32, name="ew")
            iota = pers.tile([P, P], bf16, name="iota")

            nc.sync.dma_start(hT[:], hT0_in[:])
            nc.sync.dma_start(convW[:], convW_in[:])
            nc.sync.dma_start(gWih[:], gWih_in[:])
            nc.sync.dma_start(gWhh[:], gWhh_in[:])
            nc.sync.dma_start(grub[:], grub_in[:])
            nc.sync.dma_start(lWih[:], lWih_in[:])
            nc.sync.dma_start(lWhh[:], lWhh_in[:])
            nc.sync.dma_start(lstmb[:], lstmb_in[:])
            nc.sync.dma_start(esrc[:], esrc_in[:])
            nc.sync.dma_start(eslot[:], eslot_in[:])
            nc.sync.dma_start(ew[:], ew_in[:])
            nc.sync.dma_start(iota[:], iota_in[:])

            with tc.tile_pool(name="conv", bufs=1) as convp:
                aggT = convp.tile([P, NLP], f32, name="aggT")
                m_sb = convp.tile([P, NLP], bf16, name="m_sb")

                for l in range(L):
                    # ---- 1. m_local = h @ W[l]  (node-major tiles, bf16)
                    for t in range(NB):
                        pm = pagg.tile([P, P], f32, name="pm", tag="agg128")
                        nc.tensor.matmul(pm[:], lhsT=hT[:, t * P:(t + 1) * P],
                                         rhs=convW[:, l * P:(l + 1) * P],
                                         start=True, stop=True)
                        nc.scalar.copy(out=m_sb[:, t * P:(t + 1) * P],
                                       in_=pm[:])

                    # ---- 2. DMA to bounce (node-major [NL, D]) + AllGather
                    m_bounce = dram.tile([NL, P], bf16, name=f"mb{l}")
                    m_full = dram.tile([N, P], bf16, name=f"mf{l}",
                                       addr_space="Shared")
                    m3 = m_sb[:].rearrange("p (t f) -> p t f", f=P)
                    nc.sync.dma_start(
                        m_bounce[:(NB - 1) * P, :].rearrange(
                            "(t p) f -> p t f", p=P),
                        m3[:, :NB - 1, :])
                    nc.sync.dma_start(m_bounce[(NB - 1) * P:, :],
                                      m3[:lastw, NB - 1, :])
                    nc.gpsimd.collective_compute(
                        "AllGather", ALU.bypass,
                        replica_groups=[list(range(NCORES))],
                        ins=[m_bounce[:].opt()], outs=[m_full[:].opt()])

                    # ---- 3. gather + weighted segment-sum into aggT
                    c = 0
                    for j in range(NB):
                        pj = pagg.tile([P, P], f32, name="pj", tag="agg128")
                        kj = int(cap[j])
                        for k in range(kj):
                            msg = msgp.tile([P, P], bf16, name="msg")
                            nc.gpsimd.indirect_dma_start(
                                out=msg[:], out_offset=None, in_=m_full[:],
                                in_offset=bass.IndirectOffsetOnAxis(
                                    ap=esrc[:, c:c + 1], axis=0))
                            S = sp.tile([P, P], bf16, name="S")
                            nc.vector.tensor_scalar(
                                out=S[:], in0=iota[:],
                                scalar1=eslot[:, c:c + 1],
                                scalar2=ew[:, c:c + 1],
                                op0=ALU.is_equal, op1=ALU.mult)
                            nc.tensor.matmul(pj[:], lhsT=msg[:], rhs=S[:],
                                             start=(k == 0),
                                             stop=(k == kj - 1))
                            c += 1
                        nc.scalar.copy(out=aggT[:, j * P:(j + 1) * P],
                                       in_=pj[:])

                    # ---- 4. GRU (x = aggT, h = hT) chunk by chunk, fp32
                    for (s, wdt) in chunks:
                        sl = slice(s, s + wdt)
                        pr = pbig.tile([P, 512], f32, name="pr", tag="big")
                        pz = pbig.tile([P, 512], f32, name="pz", tag="big")
                        pin = pbig.tile([P, 512], f32, name="pin", tag="big")
                        phn = pbig.tile([P, 512], f32, name="phn", tag="big")
                        for (ps, g) in ((pr, 0), (pz, 1)):
                            gs = slice(g * P, (g + 1) * P)
                            nc.tensor.matmul(ps[:, :wdt], lhsT=gWih[:, gs],
                                             rhs=aggT[:, sl],
                                             start=True, stop=False)
                            nc.tensor.matmul(ps[:, :wdt], lhsT=gWhh[:, gs],
                                             rhs=hT[:, sl],
                                             start=False, stop=True)
                        gn = slice(2 * P, 3 * P)
                        nc.tensor.matmul(pin[:, :wdt], lhsT=gWih[:, gn],
                                         rhs=aggT[:, sl],
                                         start=True, stop=True)
                        nc.tensor.matmul(phn[:, :wdt], lhsT=gWhh[:, gn],
                                         rhs=hT[:, sl],
                                         start=True, stop=True)

                        rt = tp.tile([P, 512], f32, name="rt", tag="ew1")
                        zt = tp.tile([P, 512], f32, name="zt", tag="ew2")
                        t2 = tp.tile([P, 512], f32, name="t2", tag="ew3")
                        t3 = tp.tile([P, 512], f32, name="t3", tag="ew4")
                        nt = tp.tile([P, 512], f32, name="nt", tag="ew5")
                        dd = tp.tile([P, 512], f32, name="dd", tag="ew6")
                        ee = tp.tile([P, 512], f32, name="ee", tag="ew7")
                        nc.scalar.activation(rt[:, :wdt], pr[:, :wdt],
                                             AF.Sigmoid, bias=grub[:, 0:1])
                        nc.scalar.activation(zt[:, :wdt], pz[:, :wdt],
                                             AF.Sigmoid, bias=grub[:, 1:2])
                        # t2 = (phn + bhh_n) * r
                        nc.vector.scalar_tensor_tensor(
                            out=t2[:, :wdt], in0=phn[:, :wdt],
                            scalar=grub[:, 3:4], in1=rt[:, :wdt],
                            op0=ALU.add, op1=ALU.mult)
                        nc.vector.tensor_add(t3[:, :wdt], t2[:, :wdt],
                                             pin[:, :wdt])
                        nc.scalar.activation(nt[:, :wdt], t3[:, :wdt],
                                             AF.Tanh, bias=grub[:, 2:3])
                        nc.vector.tensor_sub(dd[:, :wdt], hT[:, sl],
                                             nt[:, :wdt])
                        nc.vector.tensor_mul(ee[:, :wdt], zt[:, :wdt],
                                             dd[:, :wdt])
                        nc.vector.tensor_add(hT[:, sl], nt[:, :wdt],
                                             ee[:, :wdt])

            # ---- LSTM (x = hT fp32, hidden = HTt bf16, cell = CTt bf16)
            with tc.tile_pool(name="lstm", bufs=1) as lstmp:
                HTt = lstmp.tile([P, NLP], bf16, name="HTt")
                CTt = lstmp.tile([P, NLP], bf16, name="CTt")
                nc.sync.dma_start(HTt[:], HT_in[:])
                nc.sync.dma_start(CTt[:], CT_in[:])

                for (s, wdt) in chunks:
                    sl = slice(s, s + wdt)
                    hx = tp.tile([P, 512], bf16, name="hx", tag="ewx")
                    nc.vector.tensor_copy(hx[:, :wdt], hT[:, sl])
                    pg = [pbig.tile([P, 512], f32, name=f"pl{g}", tag="big")
                          for g in range(4)]
                    for g in range(4):
                        gs = slice(g * P, (g + 1) * P)
                        nc.tensor.matmul(pg[g][:, :wdt], lhsT=lWih[:, gs],
                                         rhs=hx[:, :wdt], start=True,
                                         stop=False)
                        nc.tensor.matmul(pg[g][:, :wdt], lhsT=lWhh[:, gs],
                                         rhs=HTt[:, sl], start=False,
                                         stop=True)
                    it = tp.tile([P, 512], f32, name="it", tag="ew1")
                    ft = tp.tile([P, 512], f32, name="ft", tag="ew2")
                    gt = tp.tile([P, 512], f32, name="gt", tag="ew3")
                    ot = tp.tile([P, 512], f32, name="ot", tag="ew4")
                    nc.scalar.activation(it[:, :wdt], pg[0][:, :wdt],
                                         AF.Sigmoid, bias=lstmb[:, 0:1])
                    nc.scalar.activation(ft[:, :wdt], pg[1][:, :wdt],
                                         AF.Sigmoid, bias=lstmb[:, 1:2])
                    nc.scalar.activation(gt[:, :wdt], pg[2][:, :wdt],
                                         AF.Tanh, bias=lstmb[:, 2:3])
                    nc.scalar.activation(ot[:, :wdt], pg[3][:, :wdt],
                                         AF.Sigmoid, bias=lstmb[:, 3:4])
                    t1 = tp.tile([P, 512], f32, name="lt1", tag="ew5")
                    t2 = tp.tile([P, 512], f32, name="lt2", tag="ew6")
                    cn = tp.tile([P, 512], f32, name="cn", tag="ew7")
                    tc_ = tp.tile([P, 512], f32, name="tcx", tag="ewt")
                    hn = tp.tile([P, 512], f32, name="hn", tag="ewh")
                    nc.vector.tensor_mul(t1[:, :wdt], ft[:, :wdt], CTt[:, sl])
                    nc.vector.tensor_mul(t2[:, :wdt], it[:, :wdt],
                                         gt[:, :wdt])
                    nc.vector.tensor_add(cn[:, :wdt], t1[:, :wdt],
                                         t2[:, :wdt])
                    nc.scalar.activation(tc_[:, :wdt], cn[:, :wdt], AF.Tanh)
                    nc.vector.tensor_mul(hn[:, :wdt], ot[:, :wdt],
                                         tc_[:, :wdt])
                    nc.sync.dma_start(Cout_ext[:, sl], cn[:, :wdt])
                    nc.sync.dma_start(Hout_ext[:, sl], hn[:, :wdt])
    return nc


_CACHE = {}


def kernel(X, edge_index, edge_weight, H, C, conv_W,
           gru_Wih, gru_Whh, gru_bih, gru_bhh,
           lstm_Wih, lstm_Whh, lstm_bih, lstm_bhh):
    X = np.asarray(X, dtype=np.float32)
    H = np.asarray(H, dtype=np.float32)
    C = np.asarray(C, dtype=np.float32)
    conv_W = np.asarray(conv_W, dtype=np.float32)
    edge_index = np.asarray(edge_index)
    edge_weight = np.asarray(edge_weight, dtype=np.float32)

    N, D = X.shape
    L = conv_W.shape[0]
    assert D == P and N % NCORES == 0
    NL = N // NCORES
    NB = (NL + P - 1) // P
    NLP = NB * P

    edata, cap, ncols = _preprocess_edges(edge_index, edge_weight, N, NL, NB)

    key = (N, D, L, ncols, tuple(cap))
    if key not in _CACHE:
        nc = _build(N, D, L, NL, NB, NLP, cap, ncols)
        nc.compile()
        _CACHE[key] = nc
    nc = _CACHE[key]

    gWihT = np.ascontiguousarray(np.asarray(gru_Wih, np.float32).T)
    gWhhT = np.ascontiguousarray(np.asarray(gru_Whh, np.float32).T)
    lWihT = np.ascontiguousarray(
        np.asarray(lstm_Wih, np.float32).T).astype(BF)
    lWhhT = np.ascontiguousarray(
        np.asarray(lstm_Whh, np.float32).T).astype(BF)
    gb = np.asarray(gru_bih, np.float32)
    gb2 = np.asarray(gru_bhh, np.float32)
    grub = np.stack([gb[0:D] + gb2[0:D], gb[D:2 * D] + gb2[D:2 * D],
                     gb[2 * D:3 * D], gb2[2 * D:3 * D]], axis=1)
    lb = np.asarray(lstm_bih, np.float32) + np.asarray(lstm_bhh, np.float32)
    lstmb = np.stack([lb[g * D:(g + 1) * D] for g in range(4)], axis=1)
    iota = np.ascontiguousarray(np.broadcast_to(
        np.arange(P, dtype=np.float32), (P, P))).astype(BF)
    convWb = np.ascontiguousarray(
        np.concatenate([conv_W[i] for i in range(L)], axis=1))

    in_maps = []
    for r in range(NCORES):
        sl = slice(r * NL, (r + 1) * NL)
        in_maps.append(dict(
            hT0=_padT(X[sl], NLP),
            HT=_padT(H[sl], NLP, BF),
            CT=_padT(C[sl], NLP, BF),
            convW=convWb, gWihT=gWihT, gWhhT=gWhhT, grub=grub,
            lWihT=lWihT, lWhhT=lWhhT, lstmb=lstmb,
            esrc=edata[r]['esrc'], eslot=edata[r]['eslot'], ew=edata[r]['ew'],
            iota=iota,
        ))

    if os.environ.get("KERNEL_SIM"):
        from concourse import bass_interp
        sim = bass_interp.MultiCoreSim(nc, NCORES)
        for r in range(NCORES):
            for k, v in in_maps[r].items():
                sim.cores[r].tensor(k)[:] = v
        sim.simulate()
        results = [{k: np.asarray(sim.cores[r].mem_tensor(k))
                    for k in ("HoutT", "CoutT")} for r in range(NCORES)]
    else:
        trace = bool(int(os.environ.get("KERNEL_TRACE", "0")))
        res = run_bass_kernel_spmd(nc, in_maps, core_ids=list(range(NCORES)),
                                   trace=trace)
        if trace:
            kernel.last_exec_time_ns = res.exec_time_ns
        results = res.results

    Hout = np.empty((N, D), dtype=np.float32)
    Cout = np.empty((N, D), dtype=np.float32)
    for r in range(NCORES):
        sl = slice(r * NL, (r + 1) * NL)
        Hout[sl] = results[r]["HoutT"].T[:NL]
        Cout[sl] = results[r]["CoutT"].T[:NL]
    return Hout, Hout, Cout


kernel.last_exec_time_ns = None
